# revision 1
# baseline (speedup 1.0000x reference)
"""Dilated self-attention Trainium2 kernel (8-core SPMD).

Problem: x[2, 8192, 1024] -> q/k/v projections -> segment-local dense
attention (SEG=512) + 4 dilated-neighbor cross-attention passes
(offsets +-1, +-2 segments, every 4th key, each with its own softmax,
weight 1/4) -> output projection.

Sharding: data-parallel over batch (2) x tensor-parallel over heads
(4 groups of 4 heads).  Each of the 8 cores runs an IDENTICAL program
on different inputs: xT for its batch, the 256-wide head-group slices
of Wq/Wk/Wv and the matching 256 rows of Wo.  Each core emits a partial
output y[8192, 1024]; the host sums the 4 head-group partials per batch.

On-core layout (all matmul operands pre-transposed so no on-device
transposes are needed):
  qT/kT  [128, 2, 8192]  features-on-partitions (head h -> chunk h//2,
                          rows (h%2)*64..)
  v5     [128, 64, 260]  tokens-on-partitions; per head 65 cols
                          [V_h (64) | ones] so each AV matmul also
                          produces the softmax denominator
  vd5    [128, 16, 260]  same for the dilated (every 4th) tokens, with
                          ones=4.0 so the denominator absorbs the
                          1/(2*NN) cross-pass weight
Scores are computed transposed (S^T[keys, q]); exp runs on ScalarE into
bf16.  AV runs in the [q, dk] orientation (lhsT = exp(S^T) chunk, rhs =
V'): out[q, j*65+64] is the softmax denominator as a per-partition
scalar, so normalization is a [128,4] reciprocal + one broadcast
multiply on the VectorE.  The accumulated [q, dk] result is moved into
the feature-major attnT layout with bf16 DMA transposes.
"""

import sys

sys.path.insert(0, "/opt/trn_rl_repo")

from contextlib import ExitStack

import numpy as np
import ml_dtypes

import concourse.tile as tile
from concourse import bacc, mybir
from concourse.bass_utils import run_bass_kernel_spmd

BF16 = mybir.dt.bfloat16
F32 = mybir.dt.float32

DIM = 1024
H = 16
DK = 64
SEG = 512
NN = 2
DIL = 4
B = 2
L = 8192
S = L // SEG            # 16 segments
HL = 4                  # heads per core
FL = HL * DK            # 256 features per core
KC = DIM // 128         # 8 contraction chunks for projections
KCS = SEG // 128        # 4 key chunks per segment (local attention)
N_CORES = 8
SCALE = 1.0 / 8.0       # 1/sqrt(DK)

_prog = None


def _build_program(repeats=1, phases=(1, 2, 3), p2_mode="full"):
    nc = bacc.Bacc(None)
    xt = nc.dram_tensor("xt", [DIM, L], BF16, kind="ExternalInput")
    wq = nc.dram_tensor("wq", [DIM, FL], BF16, kind="ExternalInput")
    wk = nc.dram_tensor("wk", [DIM, FL], BF16, kind="ExternalInput")
    wv = nc.dram_tensor("wv", [DIM, FL], BF16, kind="ExternalInput")
    wo = nc.dram_tensor("wo", [FL, DIM], BF16, kind="ExternalInput")
    y = nc.dram_tensor("y", [L, DIM], F32, kind="ExternalOutput")

    Exp = mybir.ActivationFunctionType.Exp
    Ln = mybir.ActivationFunctionType.Ln

    with tile.TileContext(nc) as tc, ExitStack() as ctx:
        singles = ctx.enter_context(tc.tile_pool(name="singles", bufs=1))
        qt = singles.tile([128, 2, L], BF16)
        kt = singles.tile([128, 2, L], BF16)
        v5 = singles.tile([128, L // 128, HL * 65], BF16)
        vd5 = singles.tile([128, (L // DIL) // 128, HL * 65], BF16)
        attnT = singles.tile([128, 2, L], BF16)
        wq_sb = singles.tile([128, KC, FL], BF16)
        wk_sb = singles.tile([128, KC, FL], BF16)
        wv_sb = singles.tile([128, KC, FL], BF16)
        wo_sb = singles.tile([128, FL // 128, DIM], BF16)
        nc.sync.dma_start(wq_sb, wq.rearrange("(k p) f -> p k f", p=128))
        nc.sync.dma_start(wk_sb, wk.rearrange("(k p) f -> p k f", p=128))
        nc.sync.dma_start(wv_sb, wv.rearrange("(k p) f -> p k f", p=128))
        nc.sync.dma_start(wo_sb, wo.rearrange("(k p) f -> p k f", p=128))
        v5_g = v5.rearrange("p c (h e) -> p c h e", e=65)
        vd5_g = vd5.rearrange("p c (h e) -> p c h e", e=65)
        nc.vector.memset(v5_g[:, :, :, 64], 1.0)
        nc.vector.memset(vd5_g[:, :, :, 64], float(2 * NN))

        for _rep in range(repeats):
          if 1 in phases:
            # ---------- Phase 1: q/k/v projections (+ dilated v) ----------
            with tc.tile_pool(name="xp", bufs=2) as xp, \
                 tc.tile_pool(name="qkps", bufs=4, space="PSUM") as qkps, \
                 tc.tile_pool(name="vps", bufs=2, space="PSUM") as vps, \
                 tc.tile_pool(name="vdps", bufs=2, space="PSUM") as vdps:
                for t in range(S):
                    sl = slice(t * SEG, (t + 1) * SEG)
                    x_t = xp.tile([128, KC, SEG], BF16)
                    nc.sync.dma_start(x_t, xt[:, sl].rearrange("(k p) n -> p k n", p=128))
                    for m in range(2):
                        for w_sb, dst in ((wq_sb, qt), (wk_sb, kt)):
                            pst = qkps.tile([128, SEG], F32, tag="qk", name="pst")
                            for k in range(KC):
                                nc.tensor.matmul(
                                    pst,
                                    w_sb[:, k, m * 128:(m + 1) * 128],
                                    x_t[:, k],
                                    start=(k == 0),
                                    stop=(k == KC - 1),
                                )
                            nc.any.tensor_copy(dst[:, m, sl], pst)
                    for sub in range(SEG // 128):
                        c = t * (SEG // 128) + sub
                        psv = vps.tile([128, FL], F32, tag="v", name="psv")
                        for k in range(KC):
                            nc.tensor.matmul(
                                psv,
                                x_t[:, k, sub * 128:(sub + 1) * 128],
                                wv_sb[:, k],
                                start=(k == 0),
                                stop=(k == KC - 1),
                            )
                        nc.any.tensor_copy(v5_g[:, c, :, 0:64], psv)
                    psvd = vdps.tile([128, FL], F32, tag="vd", name="psvd")
                    for k in range(KC):
                        nc.tensor.matmul(
                            psvd,
                            x_t[:, k, 0:SEG:DIL],
                            wv_sb[:, k],
                            start=(k == 0),
                            stop=(k == KC - 1),
                        )
                    nc.any.tensor_copy(vd5_g[:, t, :, 0:64], psvd)

          if 2 in phases:
            # ---------- Phase 2: attention ----------
            with tc.tile_pool(name="scps", bufs=2, space="PSUM") as scps, \
                 tc.tile_pool(name="avl", bufs=2, space="PSUM") as avl, \
                 tc.tile_pool(name="avx", bufs=2, space="PSUM") as avx, \
                 tc.tile_pool(name="expp", bufs=6) as expp, \
                 tc.tile_pool(name="recp", bufs=6) as recp, \
                 tc.tile_pool(name="accp", bufs=6) as accp, \
                 tc.tile_pool(name="accbf", bufs=4) as accbf:
                for s in range(S):
                    q_sl = slice(s * SEG, (s + 1) * SEG)
                    for m in range(2):
                        # local scores S^T = K^T-chunks x Q (two heads as
                        # concurrent row-tiles at partition bases 0 / 64)
                        exp_tiles = []
                        for c in range(KCS):
                            k_sl = slice(s * SEG + c * 128, s * SEG + (c + 1) * 128)
                            ps_sc = scps.tile([128, 2 * SEG], F32, tag="sc", name="ps_sc")
                            for he in range(2):
                                r0 = he * 64
                                nc.tensor.matmul(
                                    ps_sc[:, he * SEG:(he + 1) * SEG],
                                    kt[r0:r0 + 64, m, k_sl],
                                    qt[r0:r0 + 64, m, q_sl],
                                )
                            if p2_mode != "st_only":
                                e_t = expp.tile([128, 2 * SEG], BF16, tag="exp", name="e_t")
                                nc.scalar.activation(e_t, ps_sc, Exp, scale=SCALE)
                                exp_tiles.append(e_t)
                        if p2_mode in ("st_only", "st_exp"):
                            continue
                        # local AV in [q, dk] orientation; col j*65+64 is the
                        # softmax denominator (per-partition scalar)
                        accs = []
                        for he in range(2):
                            hl = 2 * m + he
                            av = avl.tile([128, 260], F32, tag="avl", name="av")
                            for c in range(KCS):
                                for j in range(4):
                                    nc.tensor.matmul(
                                        av[:, j * 65:(j + 1) * 65],
                                        exp_tiles[c][:, he * SEG + j * 128:he * SEG + (j + 1) * 128],
                                        v5[:, s * KCS + c, hl * 65:(hl + 1) * 65],
                                        start=(c == 0 and j == 0),
                                        stop=(c == KCS - 1 and j == 3),
                                    )
                            if p2_mode == "no_dve":
                                continue
                            av_g = av.rearrange("p (j e) -> p j e", e=65)
                            rec = recp.tile([128, 4], F32, tag="rec", name="rec")
                            nc.vector.reciprocal(rec, av_g[:, :, 64])
                            acc = accp.tile([128, 4, 64], F32, tag="acc", name="acc")
                            nc.vector.tensor_mul(
                                acc, av_g[:, :, 0:64],
                                rec[:, :, None].to_broadcast((128, 4, 64)),
                            )
                            accs.append(acc)
                        # dilated neighbor-segment passes
                        valid_offs = [o for o in (-2, -1, 1, 2) if 0 <= s + o < S]
                        for idx, o in enumerate(valid_offs):
                            n = s + o
                            ps_sc = scps.tile([128, 2 * SEG], F32, tag="sc", name="ps_sc")
                            for he in range(2):
                                r0 = he * 64
                                nc.tensor.matmul(
                                    ps_sc[:, he * SEG:(he + 1) * SEG],
                                    kt[r0:r0 + 64, m, n * SEG:(n + 1) * SEG:DIL],
                                    qt[r0:r0 + 64, m, q_sl],
                                )
                            e_t = expp.tile([128, 2 * SEG], BF16, tag="exp", name="e_t")
                            nc.scalar.activation(e_t, ps_sc, Exp, scale=SCALE)
                            if p2_mode == "st_exp":
                                continue
                            for he in range(2):
                                hl = 2 * m + he
                                avx_t = avx.tile([128, 260], F32, tag="avx", name="avx_t")
                                for j in range(4):
                                    nc.tensor.matmul(
                                        avx_t[:, j * 65:(j + 1) * 65],
                                        e_t[:, he * SEG + j * 128:he * SEG + (j + 1) * 128],
                                        vd5[:, n, hl * 65:(hl + 1) * 65],
                                        start=(j == 0),
                                        stop=(j == 3),
                                    )
                                if p2_mode == "no_dve":
                                    continue
                                avx_g = avx_t.rearrange("p (j e) -> p j e", e=65)
                                rec = recp.tile([128, 4], F32, tag="rec", name="rec")
                                nc.vector.reciprocal(rec, avx_g[:, :, 64])
                                tmp = accp.tile([128, 4, 64], F32, tag="tmp", name="tmp")
                                nc.vector.tensor_mul(
                                    tmp, avx_g[:, :, 0:64],
                                    rec[:, :, None].to_broadcast((128, 4, 64)),
                                )
                                nc.vector.tensor_add(accs[he], accs[he], tmp)
                        if p2_mode == "no_dve":
                            continue
                        # cast + transpose back to feature-major attnT.
                        # acc_bf packs both heads per q-slice: [q, j, he, dk],
                        # so each [128, 128] transpose lands as attnT's
                        # [he0 dk rows | he1 dk rows] block directly.
                        acc_bf = accbf.tile([128, 4, 2, 64], BF16, tag="accbf", name="acc_bf")
                        for he in range(2):
                            nc.vector.tensor_copy(acc_bf[:, :, he, :], accs[he])
                        for j in range(4):
                            nc.sync.dma_start_transpose(
                                attnT[:, m, s * SEG + j * 128:s * SEG + (j + 1) * 128],
                                acc_bf[:, j, :, :],
                            )

          if 3 in phases:
            # ---------- Phase 3: output projection ----------
            with tc.tile_pool(name="yps", bufs=3, space="PSUM") as yps, \
                 tc.tile_pool(name="ysb", bufs=3) as ysb:
                for tcn in range(L // 128):
                    ps_y = yps.tile([128, DIM], F32, tag="y", name="ps_y")
                    for nh in range(2):
                        for m in range(2):
                            nc.tensor.matmul(
                                ps_y[:, nh * 512:(nh + 1) * 512],
                                attnT[:, m, tcn * 128:(tcn + 1) * 128],
                                wo_sb[:, m, nh * 512:(nh + 1) * 512],
                                start=(m == 0),
                                stop=(m == 1),
                            )
                    y_t = ysb.tile([128, DIM], F32, tag="ysb", name="y_t")
                    nc.any.tensor_copy(y_t, ps_y)
                    nc.sync.dma_start(y[tcn * 128:(tcn + 1) * 128, :], y_t)

    nc.compile()
    return nc


def _make_in_maps(x, Wq, Wk, Wv, Wo):
    bf = ml_dtypes.bfloat16
    xt_b = [np.asarray(x[b]).T.astype(bf) for b in range(B)]
    wq_g = [np.asarray(Wq[:, g * FL:(g + 1) * FL]).astype(bf) for g in range(4)]
    wk_g = [np.asarray(Wk[:, g * FL:(g + 1) * FL]).astype(bf) for g in range(4)]
    wv_g = [np.asarray(Wv[:, g * FL:(g + 1) * FL]).astype(bf) for g in range(4)]
    wo_g = [np.asarray(Wo[g * FL:(g + 1) * FL, :]).astype(bf) for g in range(4)]
    in_maps = []
    for c in range(N_CORES):
        b, g = divmod(c, 4)
        in_maps.append(
            {"xt": xt_b[b], "wq": wq_g[g], "wk": wk_g[g], "wv": wv_g[g],
             "wo": wo_g[g]}
        )
    return in_maps


def run(x, Wq, bq, Wk, bk, Wv, bv, Wo, bo, trace=False, tmpdir=None):
    """Build (cached), run on 8 cores, gather. Returns (y, BassKernelResults)."""
    global _prog
    if _prog is None:
        _prog = _build_program()
    in_maps = _make_in_maps(x, Wq, Wk, Wv, Wo)
    res = run_bass_kernel_spmd(
        _prog, in_maps, core_ids=list(range(N_CORES)), trace=trace, tmpdir=tmpdir
    )
    y = np.zeros((B, L, DIM), np.float32)
    for c in range(N_CORES):
        y[c // 4] += res.results[c]["y"]
    # bq/bk/bv are identically zero in this problem; bo is added on host.
    y += np.asarray(bo, np.float32)[None, None, :]
    return y, res


def kernel(x, Wq, bq, Wk, bk, Wv, bv, Wo, bo):
    y, _ = run(x, Wq, bq, Wk, bk, Wv, bv, Wo, bo)
    return y

